# revision 1
# baseline (speedup 1.0000x reference)
"""Tensor-parallel attention kernel for Trainium2 (8 NeuronCores).

Problem: B=1, L=2048, D=4096, H=32 q-heads, KV=8 kv-heads, HD=128,
partial rotary ROT=64, causal additive mask, o-projection.

Sharding: TP-8 over heads. Core c owns q-heads 4c..4c+3 and kv-head c
(column shard of w_qkv), plus the matching row shard of w_o. Each core
computes a full [L, D] partial of the output; the host sums the 8
partials (the cross-core reduction of the row-sharded o-projection).

Everything on-chip runs in "transposed" orientation so every matmul
contracts over the partition dim with zero on-chip activation
transposes:
  qkvT[col, L] = w_qkv.T @ x.T          (w stationary, xT streamed)
  rope:  qT' = qT * cosE + (P @ qT) * sinE   (P = rotate-half matrix on PE)
  ST[k, q]   = kT_tile.T @ qT            (one matmul per k-tile, K=HD=128)
  PT         = exp(ST + maskT)           (no max subtraction; exp(-1e9)=0)
  den[*, q]  = ones.T @ PT               (ones-matmul, accumulated over k)
  oT[d, q]   = V_tile.T @ PT             (V from a one-time PE transpose of vT)
  out[l, e]  = (oT/den).T @ w_o_shard    (partial; summed across cores on host)
"""

import sys

for _p in ("/opt/trn_rl_repo", "/root/.axon_site/_ro/trn_rl_repo"):
    if _p not in sys.path:
        sys.path.append(_p)

import numpy as np

B, L, D = 1, 2048, 4096
H, KV, HD = 32, 8, 128
ROT = 64
SCALE = HD ** -0.5
NEG = -1e9
NCORES = 8
HPC = H // NCORES          # q-heads per core (4)
CPC = HPC * HD + 2 * HD    # w_qkv columns per core (768)
NDT = D // 128             # contraction tiles over D (32)
NKT = L // 128             # k tiles (16)
NJQ = L // 512             # 512-wide q blocks (4)
XBLK = 256                 # L-block width in the qkv phase

_cache = {}


def _build(causal: bool):
    import concourse.mybir as mybir
    import concourse.tile as tile
    from concourse import bacc

    F32 = mybir.dt.float32
    F32R = mybir.dt.float32r
    EXP = mybir.ActivationFunctionType.Exp

    nc = bacc.Bacc("TRN2", target_bir_lowering=False, debug=False)

    xt = nc.dram_tensor("xt", [D, L], F32, kind="ExternalInput").ap()
    wqkv = nc.dram_tensor("wqkv", [D, CPC], F32, kind="ExternalInput").ap()
    wo = nc.dram_tensor("wo", [HPC * HD, D], F32, kind="ExternalInput").ap()
    cos_e = nc.dram_tensor("cos_e", [2, 128, L], F32, kind="ExternalInput").ap()
    sin_e = nc.dram_tensor("sin_e", [2, 128, L], F32, kind="ExternalInput").ap()
    consts = nc.dram_tensor("consts", [128, 384], F32, kind="ExternalInput").ap()
    if causal:
        # block-diagonal strip of maskT: [jq, ktile-in-block, 128, 512]
        mask_d = nc.dram_tensor("mask_d", [NJQ, 4, 128, 512], F32,
                                kind="ExternalInput").ap()
    else:
        mask_t = nc.dram_tensor("mask_t", [L, L], F32, kind="ExternalInput").ap()
    out_p = nc.dram_tensor("out_p", [L, D], F32, kind="ExternalOutput").ap()

    qt_r = nc.dram_tensor("qt_r", [HPC, 128, L], F32R)  # roped qT, internal

    xt_r = xt.rearrange("(dt p) l -> p dt l", p=128).bitcast(F32R)
    wqkv_r = wqkv.rearrange("(dt p) c -> p dt c", p=128).bitcast(F32R)
    wo_r = wo.rearrange("(h p) e -> p h e", p=128).bitcast(F32R)

    with tile.TileContext(nc) as tc:
        with tc.tile_pool(name="persist", bufs=1) as persist:
            kt_sb = persist.tile([128, L], F32R, tag="kt")
            v_sb = persist.tile([128, NKT, 128], F32R, tag="v")
            cst = persist.tile([128, 384], F32R, tag="cst")
            nc.sync.dma_start(out=cst, in_=consts.bitcast(F32R))
            ident = cst[:, 0:128]
            ones = cst[:, 128:256]
            pmat_t = cst[:, 256:384]

            # ---------------- Phase 1: qkv projection + rope ----------------
            with tc.tile_pool(name="wq", bufs=1) as wqp, \
                 tc.tile_pool(name="xb", bufs=2) as xbp, \
                 tc.tile_pool(name="tabs", bufs=1) as tabs, \
                 tc.tile_pool(name="stage", bufs=3) as stage, \
                 tc.tile_pool(name="vtmp", bufs=1) as vtmp, \
                 tc.tile_pool(name="ps1", bufs=4, space="PSUM") as ps1, \
                 tc.tile_pool(name="psr", bufs=2, space="PSUM") as psr:
                wq_sb = wqp.tile([128, NDT, CPC], F32R)
                vt_sb = vtmp.tile([128, L], F32R)

                for lb in range(L // XBLK):
                    ls = slice(lb * XBLK, (lb + 1) * XBLK)
                    xblk = xbp.tile([128, NDT, XBLK], F32R, tag="xblk")
                    if lb == 0:
                        # interleave first x chunks with weight slabs so the
                        # first accumulation can start as data arrives
                        for sl in range(8):
                            ss = slice(sl * NDT // 8, (sl + 1) * NDT // 8)
                            nc.gpsimd.dma_start(out=xblk[:, ss, :], in_=xt_r[:, ss, ls])
                            weng = nc.sync if sl % 2 == 0 else nc.scalar
                            weng.dma_start(out=wq_sb[:, ss, :], in_=wqkv_r[:, ss, :])
                    else:
                        nc.gpsimd.dma_start(out=xblk, in_=xt_r[:, :, ls])
                    cosb = tabs.tile([128, 2, XBLK], F32, tag="cosb")
                    sinb = tabs.tile([128, 2, XBLK], F32, tag="sinb")
                    nc.sync.dma_start(out=cosb, in_=cos_e[:, :, ls].rearrange("t p l -> p t l"))
                    nc.sync.dma_start(out=sinb, in_=sin_e[:, :, ls].rearrange("t p l -> p t l"))
                    for ct in range(6):
                        acc = ps1.tile([128, XBLK], F32, tag="acc")
                        for dti in range(NDT):
                            nc.tensor.matmul(
                                out=acc,
                                lhsT=wq_sb[:, dti, ct * 128:(ct + 1) * 128],
                                rhs=xblk[:, dti, :],
                                start=(dti == 0), stop=(dti == NDT - 1))
                        if ct == 5:
                            # v: copy to vT staging, then transpose this
                            # block's two k-tiles into resident V
                            nc.scalar.copy(out=vt_sb[:, ls], in_=acc)
                            for kk in range(2):
                                i = 2 * lb + kk
                                tp = psr.tile([128, 128], F32R, tag="vtp")
                                nc.tensor.transpose(
                                    tp, vt_sb[:, i * 128:(i + 1) * 128], ident)
                                nc.vector.tensor_copy(v_sb[:, i, :], tp)
                            continue
                        # rope for q (ct 0..3, scaled tables) and k (ct 4)
                        ti = 0 if ct < 4 else 1
                        s_sb = stage.tile([128, XBLK], F32R, tag="s_sb")
                        nc.scalar.copy(out=s_sb, in_=acc)
                        rot = psr.tile([128, XBLK], F32, tag="rot")
                        nc.tensor.matmul(out=rot, lhsT=pmat_t, rhs=s_sb,
                                         start=True, stop=True)
                        dst = kt_sb[:, ls] if ct == 4 else None
                        if dst is None:
                            dtile = stage.tile([128, XBLK], F32R, tag="dtile")
                        else:
                            dtile = dst
                        nc.vector.tensor_mul(dtile, s_sb, cosb[:, ti, :])
                        m2 = stage.tile([128, XBLK], F32R, tag="m2")
                        nc.vector.tensor_mul(m2, rot, sinb[:, ti, :])
                        nc.vector.tensor_add(dtile, dtile, m2)
                        if dst is None:
                            nc.sync.dma_start(out=qt_r[ct][:, ls], in_=dtile)


            # ---------------- Phases 2+3 ----------------
            late_cm = tc.tile_pool(name="late", bufs=1)
            late = late_cm.__enter__()
            otn_sb = late.tile([128, HPC, L], F32R, tag="otn")

            # ---------------- Phase 2: attention ----------------
            with tc.tile_pool(name="qb", bufs=3) as qbp, \
                 tc.tile_pool(name="mb", bufs=2) as mbp, \
                 tc.tile_pool(name="pt", bufs=6) as ptp, \
                 tc.tile_pool(name="rdp", bufs=2) as rdp, \
                 tc.tile_pool(name="ps_st", bufs=4, space="PSUM") as ps_st, \
                 tc.tile_pool(name="ps_acc", bufs=2, space="PSUM") as ps_acc:
                for jq in range(NJQ):
                    qs = slice(jq * 512, (jq + 1) * 512)
                    nkt = 4 * (jq + 1) if causal else NKT
                    diag0 = 4 * jq
                    if causal:
                        mblk = mbp.tile([128, 4, 512], F32, tag="mblk")
                        nc.sync.dma_start(
                            out=mblk, in_=mask_d[jq].rearrange("kt p q -> p kt q"))
                    else:
                        mblk = mbp.tile([128, NKT, 512], F32, tag="mblk")
                        nc.sync.dma_start(
                            out=mblk,
                            in_=mask_t[:, qs].rearrange("(kt p) q -> p kt q", p=128))
                    for h in range(HPC):
                        qblk = qbp.tile([128, 512], F32R, tag="qblk")
                        nc.sync.dma_start(out=qblk, in_=qt_r[h][:, qs])
                        den = ps_acc.tile([128, 512], F32, tag="den")
                        ot = ps_acc.tile([128, 512], F32, tag="ot")
                        for i in range(nkt):
                            st = ps_st.tile([128, 512], F32, tag="st")
                            nc.tensor.matmul(
                                out=st, lhsT=kt_sb[:, i * 128:(i + 1) * 128],
                                rhs=qblk, start=True, stop=True)
                            if causal:
                                if i >= diag0:
                                    nc.vector.tensor_add(st, st, mblk[:, i - diag0, :])
                            else:
                                nc.vector.tensor_add(st, st, mblk[:, i, :])
                            pt = ptp.tile([128, 512], F32R, tag="pt")
                            nc.scalar.activation(pt, st, EXP)
                            nc.tensor.matmul(out=den, lhsT=ones, rhs=pt,
                                             start=(i == 0), stop=(i == nkt - 1))
                            nc.tensor.matmul(out=ot, lhsT=v_sb[:, i, :], rhs=pt,
                                             start=(i == 0), stop=(i == nkt - 1))
                        rd = rdp.tile([128, 512], F32, tag="rd")
                        nc.vector.reciprocal_approx_fast(out=rd, in_=den)
                        nc.vector.tensor_mul(otn_sb[:, h, qs], ot, rd)


            # ---------------- Phase 3: o-projection ----------------
            with tc.tile_pool(name="wob", bufs=2) as wop, \
                 tc.tile_pool(name="ost", bufs=6) as ostp, \
                 tc.tile_pool(name="ps3", bufs=6, space="PSUM") as ps3:
                for et in range(D // 512):
                    es = slice(et * 512, (et + 1) * 512)
                    wob = wop.tile([128, HPC, 512], F32R, tag="wob")
                    nc.gpsimd.dma_start(out=wob, in_=wo_r[:, :, es])
                    for lt in range(L // 128):
                        acc = ps3.tile([128, 512], F32, tag="acc3")
                        for h in range(HPC):
                            nc.tensor.matmul(
                                out=acc,
                                lhsT=otn_sb[:, h, lt * 128:(lt + 1) * 128],
                                rhs=wob[:, h, :],
                                start=(h == 0), stop=(h == HPC - 1))
                        ost = ostp.tile([128, 512], F32, tag="ost")
                        if lt % 2 == 0:
                            nc.vector.tensor_copy(ost, acc)
                        else:
                            nc.scalar.copy(out=ost, in_=acc)
                        nc.sync.dma_start(out=out_p[lt * 128:(lt + 1) * 128, es], in_=ost)

            late_cm.__exit__(None, None, None)

    nc.compile()
    return nc


def _host_inputs(x, attention_mask, cos, sin, w_qkv, w_o, causal):
    """Build the 8 per-core input maps (all fp32, C-contiguous)."""
    xt = np.ascontiguousarray(x[0].T)                     # [D, L]
    q_pos = H * HD
    kv_pos = q_pos + KV * HD

    # extended rope tables [2, 128, L]: slot 0 = q (scale folded), slot 1 = k
    # row d<64: cos[l, d]; row d>=64: 1.0 (cos) / 0.0 (sin)
    cos_t = cos.T.astype(np.float32)                      # [ROT, L]
    sin_t = sin.T.astype(np.float32)
    cos_e = np.empty((2, 128, L), np.float32)
    sin_e = np.zeros((2, 128, L), np.float32)
    cos_e[0, :ROT] = cos_t * SCALE
    cos_e[0, ROT:] = SCALE
    cos_e[1, :ROT] = cos_t
    cos_e[1, ROT:] = 1.0
    sin_e[0, :ROT] = sin_t * SCALE
    sin_e[1, :ROT] = sin_t

    # consts [128, 384] = [identity | ones | pmat_t]
    # pmat_t[d, d'] = Pmat[d', d]; rot[d'] = -x[d'+32] (d'<32), x[d'-32] (32<=d'<64)
    pmat = np.zeros((128, 128), np.float32)
    for dp in range(32):
        pmat[dp, dp + 32] = -1.0
    for dp in range(32, 64):
        pmat[dp, dp - 32] = 1.0
    consts = np.concatenate(
        [np.eye(128, dtype=np.float32), np.ones((128, 128), np.float32), pmat.T], axis=1)

    mask2d = np.ascontiguousarray(attention_mask[0, 0])   # [L(q), L(k)]
    if causal:
        mask_t_full = None
        # diagonal 512x512 blocks of maskT, split into 128-row k strips
        mask_d = np.empty((NJQ, 4, 128, 512), np.float32)
        mt = mask2d.T                                     # [k, q]
        for jq in range(NJQ):
            blk = mt[jq * 512:(jq + 1) * 512, jq * 512:(jq + 1) * 512]
            mask_d[jq] = blk.reshape(4, 128, 512)
        mask_d = np.ascontiguousarray(mask_d)
    else:
        mask_t_full = np.ascontiguousarray(mask2d.T)      # [k, q]
        mask_d = None

    in_maps = []
    for c in range(NCORES):
        cols = []
        for j in range(HPC):
            h = c * HPC + j
            cols.append(w_qkv[:, h * HD:(h + 1) * HD])
        cols.append(w_qkv[:, q_pos + c * HD:q_pos + (c + 1) * HD])
        cols.append(w_qkv[:, kv_pos + c * HD:kv_pos + (c + 1) * HD])
        wqkv_c = np.ascontiguousarray(np.concatenate(cols, axis=1))  # [D, 768]
        wo_c = np.ascontiguousarray(
            w_o[c * HPC * HD:(c + 1) * HPC * HD, :])                 # [512, D]
        m = {"xt": xt, "wqkv": wqkv_c, "wo": wo_c,
             "cos_e": cos_e, "sin_e": sin_e, "consts": consts}
        if causal:
            m["mask_d"] = mask_d
        else:
            m["mask_t"] = mask_t_full
        in_maps.append(m)
    return in_maps


def _is_causal(mask2d):
    expected = np.where(
        np.tril(np.ones((L, L), dtype=bool)), np.float32(0.0), np.float32(NEG))
    return mask2d.shape == (L, L) and np.array_equal(mask2d, expected)


def kernel(x, attention_mask, cos, sin, w_qkv, w_o, _trace=False):
    from concourse.bass_utils import run_bass_kernel_spmd

    x = np.asarray(x, dtype=np.float32)
    attention_mask = np.asarray(attention_mask, dtype=np.float32)
    cos = np.asarray(cos, dtype=np.float32)
    sin = np.asarray(sin, dtype=np.float32)
    w_qkv = np.asarray(w_qkv, dtype=np.float32)
    w_o = np.asarray(w_o, dtype=np.float32)

    causal = _is_causal(attention_mask[0, 0])
    if causal not in _cache:
        _cache[causal] = _build(causal)
    nc = _cache[causal]

    in_maps = _host_inputs(x, attention_mask, cos, sin, w_qkv, w_o, causal)
    try:
        res = run_bass_kernel_spmd(nc, in_maps, list(range(NCORES)), trace=_trace)
    except Exception:
        # transient device errors (e.g. NRT_EXEC_UNIT_UNRECOVERABLE) usually
        # clear on retry
        res = run_bass_kernel_spmd(nc, in_maps, list(range(NCORES)), trace=_trace)
    out = np.zeros((L, D), np.float64)
    for c in range(NCORES):
        out += res.results[c]["out_p"].astype(np.float64)
    if _trace:
        kernel._last_exec_time_ns = res.exec_time_ns
    return out.astype(np.float32).reshape(B, L, D)



# revision 3
# speedup vs baseline: 1.1038x; 1.1038x over previous
"""Tensor-parallel attention kernel for Trainium2 (8 NeuronCores).

Problem: B=1, L=2048, D=4096, H=32 q-heads, KV=8 kv-heads, HD=128,
partial rotary ROT=64, causal additive mask, o-projection.

Sharding: TP-8 over heads. Core c owns q-heads 4c..4c+3 and kv-head c
(column shard of w_qkv), plus the matching row shard of w_o. Each core
computes a full [L, D] partial of the output; the host sums the 8
partials (the cross-core reduction of the row-sharded o-projection).

All on-chip data is bf16 (PSUM accumulation stays f32): halves HBM
traffic vs f32 and lets the PE use FWL weight loads. rel-err budget is
2e-2; bf16 end-to-end measures ~4e-3.

Everything runs in "transposed" orientation so every matmul contracts
over the partition dim with zero on-chip activation transposes:
  qkvT[col, L] = w_qkv.T @ x.T          (w stationary, xT streamed)
  rope:  qT' = qT * cosE + (P @ qT) * sinE   (P = rotate-half matrix on PE)
  ST[k, q]   = kT_tile.T @ qT            (one matmul per k-tile, K=HD=128)
  PT         = exp(ST + tri)             (exp batched over k-tile pairs)
  den[*, q]  = ones.T @ PT               (ones-matmul, accumulated over k)
  oT[d, q]   = V_tile.T @ PT             (V from a one-time PE transpose of vT)
  out[l, e]  = (oT/den).T @ w_o_shard    (partial; summed across cores on host)

Causality is exploited at 128-row k-strip granularity: strips fully
below the diagonal skip the mask entirely; diagonal strips compute only
the valid q-subrange and add a single shared [128,128] triangle mask.
"""

import sys

for _p in ("/opt/trn_rl_repo", "/root/.axon_site/_ro/trn_rl_repo"):
    if _p not in sys.path:
        sys.path.append(_p)

import numpy as np
import ml_dtypes

B, L, D = 1, 2048, 4096
H, KV, HD = 32, 8, 128
ROT = 64
SCALE = HD ** -0.5
NEG = -1e9
NCORES = 8
HPC = H // NCORES          # q-heads per core (4)
CPC = HPC * HD + 2 * HD    # w_qkv columns per core (768)
NDT = D // 128             # contraction tiles over D (32)
NKT = L // 128             # k tiles (16)
NJQ = L // 512             # 512-wide q blocks (4)
XBLK = 512                 # L-block width in the qkv phase
NLB = L // XBLK            # 4

NPBF16 = ml_dtypes.bfloat16

_cache = {}


def _build(causal: bool):
    import concourse.mybir as mybir
    import concourse.tile as tile
    from concourse import bacc

    F32 = mybir.dt.float32
    BF16 = mybir.dt.bfloat16
    EXP = mybir.ActivationFunctionType.Exp

    nc = bacc.Bacc("TRN2", target_bir_lowering=False, debug=False)

    xt = nc.dram_tensor("xt", [D, L], BF16, kind="ExternalInput").ap()
    wqkv = nc.dram_tensor("wqkv", [D, CPC], BF16, kind="ExternalInput").ap()
    wo = nc.dram_tensor("wo", [HPC * HD, D], BF16, kind="ExternalInput").ap()
    cos_e = nc.dram_tensor("cos_e", [2, 128, L], F32, kind="ExternalInput").ap()
    sin_e = nc.dram_tensor("sin_e", [2, 128, L], F32, kind="ExternalInput").ap()
    consts = nc.dram_tensor("consts", [128, 384], BF16, kind="ExternalInput").ap()
    tri_t = nc.dram_tensor("tri_t", [128, 128], F32, kind="ExternalInput").ap()
    if not causal:
        mask_t = nc.dram_tensor("mask_t", [L, L], F32, kind="ExternalInput").ap()
    out_p = nc.dram_tensor("out_p", [L, D], BF16, kind="ExternalOutput").ap()

    xt_r = xt.rearrange("(dt p) l -> p dt l", p=128)
    wqkv_r = wqkv.rearrange("(dt p) c -> p dt c", p=128)
    wo_r = wo.rearrange("(h p) e -> p h e", p=128)

    with tile.TileContext(nc) as tc:
        with tc.tile_pool(name="persist", bufs=1) as persist:
            kt_sb = persist.tile([128, L], BF16, tag="kt")
            v_sb = persist.tile([128, NKT, 128], BF16, tag="v")
            qt_sb = persist.tile([128, HPC, L], BF16, tag="qt")
            wo_sb = persist.tile([128, HPC, D], BF16, tag="wo")
            cst = persist.tile([128, 384], BF16, tag="cst")
            tri = persist.tile([128, 128], F32, tag="tri")
            nc.sync.dma_start(out=cst, in_=consts)
            nc.sync.dma_start(out=tri, in_=tri_t)
            ident = cst[:, 0:128]
            ones = cst[:, 128:256]
            pmat_t = cst[:, 256:384]

            # ---------------- Phase 1: qkv projection + rope ----------------
            with tc.tile_pool(name="wq", bufs=1) as wqp, \
                 tc.tile_pool(name="xb", bufs=2) as xbp, \
                 tc.tile_pool(name="tabs", bufs=2) as tabs, \
                 tc.tile_pool(name="stage", bufs=3) as stage, \
                 tc.tile_pool(name="vtmp", bufs=2) as vtmp, \
                 tc.tile_pool(name="ps1", bufs=4, space="PSUM") as ps1, \
                 tc.tile_pool(name="psr", bufs=2, space="PSUM") as psr, \
                 tc.tile_pool(name="pst", bufs=2, space="PSUM") as pstp:
                wq_sb = wqp.tile([128, NDT, CPC], BF16)

                # weight DMA in dti-quarters on gpsimd; wo preload after
                for qtr in range(4):
                    qs_ = slice(8 * qtr, 8 * qtr + 8)
                    nc.gpsimd.dma_start(out=wq_sb[:, qs_, :], in_=wqkv_r[:, qs_, :])
                nc.gpsimd.dma_start(out=wo_sb, in_=wo_r)

                def rope_ct(lb, ct, acc, tb):
                    """Consume psum acc for column-group ct of L-block lb."""
                    ls = slice(lb * XBLK, (lb + 1) * XBLK)
                    if ct == 5:
                        # v: stage, then PE-transpose this block's k-tiles
                        vt_sb = vtmp.tile([128, XBLK], BF16, tag="vt")
                        nc.scalar.copy(out=vt_sb, in_=acc)
                        for kk in range(XBLK // 128):
                            i = lb * (XBLK // 128) + kk
                            tp = pstp.tile([128, 128], BF16, tag="vtp")
                            nc.tensor.transpose(
                                tp, vt_sb[:, kk * 128:(kk + 1) * 128], ident)
                            nc.vector.tensor_copy(v_sb[:, i, :], tp)
                        return
                    # rope for q (ct 0..3, scaled tables) and k (ct 4)
                    ti = 0 if ct < 4 else 1
                    s_sb = stage.tile([128, XBLK], BF16, tag="s_sb")
                    nc.scalar.copy(out=s_sb, in_=acc)
                    rot = psr.tile([128, XBLK], F32, tag="rot")
                    nc.tensor.matmul(out=rot, lhsT=pmat_t, rhs=s_sb,
                                     start=True, stop=True)
                    dst = qt_sb[:, ct, ls] if ct < 4 else kt_sb[:, ls]
                    m2 = stage.tile([128, XBLK], BF16, tag="m2")
                    # all-f32 inputs, bf16 outputs (cast on write)
                    nc.vector.tensor_mul(dst, acc, tb[:, ti, :])
                    nc.vector.tensor_mul(m2, rot, tb[:, 2 + ti, :])
                    nc.vector.tensor_add(dst, dst, m2)

                for lb in range(NLB):
                    ls = slice(lb * XBLK, (lb + 1) * XBLK)
                    xblk = xbp.tile([128, NDT, XBLK], BF16, tag="xblk")
                    if lb == 0:
                        for qtr in range(4):
                            qs_ = slice(8 * qtr, 8 * qtr + 8)
                            nc.sync.dma_start(out=xblk[:, qs_, :],
                                              in_=xt_r[:, qs_, ls])
                    else:
                        nc.sync.dma_start(out=xblk, in_=xt_r[:, :, ls])
                    tb = tabs.tile([128, 4, XBLK], F32, tag="tb")
                    nc.scalar.dma_start(
                        out=tb[:, 0:2, :],
                        in_=cos_e[:, :, ls].rearrange("t p l -> p t l"))
                    nc.scalar.dma_start(
                        out=tb[:, 2:4, :],
                        in_=sin_e[:, :, ls].rearrange("t p l -> p t l"))
                    if lb == 0:
                        # two passes of 3 column-groups, dti-quarter inner, so
                        # the first matmuls only wait on the first x/w quarter
                        accs = {}
                        for half in range(2):
                            cts = range(3 * half, 3 * half + 3)
                            for ct in cts:
                                accs[ct] = ps1.tile(
                                    [128, XBLK], F32, tag="acc",
                                    name=f"acc_l0_{ct}")
                            for qtr in range(4):
                                for ct in cts:
                                    for dti in range(8 * qtr, 8 * qtr + 8):
                                        nc.tensor.matmul(
                                            out=accs[ct],
                                            lhsT=wq_sb[:, dti,
                                                       ct * 128:(ct + 1) * 128],
                                            rhs=xblk[:, dti, :],
                                            start=(dti == 0),
                                            stop=(dti == NDT - 1))
                            for ct in cts:
                                rope_ct(lb, ct, accs[ct], tb)
                    else:
                        for ct in range(6):
                            acc = ps1.tile([128, XBLK], F32, tag="acc")
                            for dti in range(NDT):
                                nc.tensor.matmul(
                                    out=acc,
                                    lhsT=wq_sb[:, dti, ct * 128:(ct + 1) * 128],
                                    rhs=xblk[:, dti, :],
                                    start=(dti == 0), stop=(dti == NDT - 1))
                            rope_ct(lb, ct, acc, tb)

            # ---------------- Phases 2+3 ----------------
            late_cm = tc.tile_pool(name="late", bufs=1)
            late = late_cm.__enter__()
            otn_sb = late.tile([128, HPC, L], BF16, tag="otn")

            # ---------------- Phase 2: attention ----------------
            with tc.tile_pool(name="mb", bufs=2) as mbp, \
                 tc.tile_pool(name="pt", bufs=3) as ptp, \
                 tc.tile_pool(name="rdp", bufs=2) as rdp, \
                 tc.tile_pool(name="ps_st", bufs=2, space="PSUM") as ps_st, \
                 tc.tile_pool(name="ps_acc", bufs=2, space="PSUM") as ps_acc:
                for jq in range(NJQ):
                    qs = slice(jq * 512, (jq + 1) * 512)
                    if not causal:
                        mblk = mbp.tile([128, NKT, 512], F32, tag="mblk")
                        nc.scalar.dma_start(
                            out=mblk,
                            in_=mask_t[:, qs].rearrange("(kt p) q -> p kt q",
                                                        p=128))
                    for h in range(HPC):
                        den = ps_acc.tile([128, 512], F32, tag="den")
                        ot = ps_acc.tile([128, 512], F32, tag="ot")
                        # each pair entry: (pt2, i2 -> (ktile, dstart, width))
                        pend = None
                        first = [True]

                        def flush(pend):
                            if pend is None:
                                return
                            pt2, specs = pend
                            for i2, (i, ds, w) in enumerate(specs):
                                st_ = first[0] and i2 == 0
                                sp_ = (i == NKT - 1) if not causal else \
                                    (i == 4 * jq + 3)
                                nc.tensor.matmul(
                                    out=den[:, ds:ds + w], lhsT=ones[:, 0:128],
                                    rhs=pt2[:, i2, 0:w],
                                    start=st_, stop=sp_,
                                    skip_group_check=causal)
                                nc.tensor.matmul(
                                    out=ot[:, ds:ds + w], lhsT=v_sb[:, i, :],
                                    rhs=pt2[:, i2, 0:w],
                                    start=st_, stop=sp_,
                                    skip_group_check=causal)
                            first[0] = False

                        nfull = 4 * jq if causal else NKT
                        ndiag = 4 if causal else 0
                        npairs = (nfull + ndiag) // 2
                        for m in range(npairs):
                            st2 = ps_st.tile([128, 2, 512], F32, tag="st2")
                            pt2 = ptp.tile([128, 2, 512], BF16, tag="pt2")
                            specs = []
                            for i2 in range(2):
                                i = 2 * m + i2
                                if i < nfull:
                                    ds, w = 0, 512
                                    qsl = qs
                                else:
                                    d = i - nfull
                                    ds, w = 128 * d, 512 - 128 * d
                                    qsl = slice(jq * 512 + 128 * d,
                                                (jq + 1) * 512)
                                nc.tensor.matmul(
                                    out=st2[:, i2, 0:w],
                                    lhsT=kt_sb[:, i * 128:(i + 1) * 128],
                                    rhs=qt_sb[:, h, qsl],
                                    start=True, stop=True)
                                if i >= nfull:
                                    nc.vector.tensor_add(
                                        st2[:, i2, 0:128], st2[:, i2, 0:128],
                                        tri)
                                elif not causal:
                                    nc.vector.tensor_add(
                                        st2[:, i2, :], st2[:, i2, :],
                                        mblk[:, i, :])
                                specs.append((i, ds, w))
                            nc.scalar.activation(pt2, st2, EXP)
                            flush(pend)
                            pend = (pt2, specs)
                        flush(pend)
                        rd = rdp.tile([128, 512], F32, tag="rd")
                        nc.vector.reciprocal_approx_fast(out=rd, in_=den)
                        nc.vector.tensor_mul(otn_sb[:, h, qs], ot, rd)

            # ---------------- Phase 3: o-projection ----------------
            with tc.tile_pool(name="ost", bufs=2) as ostp, \
                 tc.tile_pool(name="ps3", bufs=4, space="PSUM") as ps3:
                for lt in range(L // 128):
                    ost = ostp.tile([128, D], BF16, tag="ost")
                    for et in range(D // 512):
                        es = slice(et * 512, (et + 1) * 512)
                        acc = ps3.tile([128, 512], F32, tag="acc3")
                        for h in range(HPC):
                            nc.tensor.matmul(
                                out=acc,
                                lhsT=otn_sb[:, h, lt * 128:(lt + 1) * 128],
                                rhs=wo_sb[:, h, es],
                                start=(h == 0), stop=(h == HPC - 1))
                        if et % 2 == 0:
                            nc.vector.tensor_copy(ost[:, es], acc)
                        else:
                            nc.scalar.copy(out=ost[:, es], in_=acc)
                    nc.sync.dma_start(
                        out=out_p[lt * 128:(lt + 1) * 128, :], in_=ost)

            late_cm.__exit__(None, None, None)

    nc.compile()
    return nc


def _host_inputs(x, attention_mask, cos, sin, w_qkv, w_o, causal):
    """Build the 8 per-core input maps (bf16 data, C-contiguous)."""
    xt = np.ascontiguousarray(x[0].T).astype(NPBF16)      # [D, L]
    q_pos = H * HD
    kv_pos = q_pos + KV * HD

    # extended rope tables [2, 128, L]: slot 0 = q (scale folded), slot 1 = k
    # row d<64: cos[l, d]; row d>=64: 1.0 (cos) / 0.0 (sin)
    cos_t = cos.T.astype(np.float32)                      # [ROT, L]
    sin_t = sin.T.astype(np.float32)
    cos_e = np.empty((2, 128, L), np.float32)
    sin_e = np.zeros((2, 128, L), np.float32)
    cos_e[0, :ROT] = cos_t * SCALE
    cos_e[0, ROT:] = SCALE
    cos_e[1, :ROT] = cos_t
    cos_e[1, ROT:] = 1.0
    sin_e[0, :ROT] = sin_t * SCALE
    sin_e[1, :ROT] = sin_t

    # consts [128, 384] = [identity | ones | pmat_t] (bf16)
    pmat = np.zeros((128, 128), np.float32)
    for dp in range(32):
        pmat[dp, dp + 32] = -1.0
    for dp in range(32, 64):
        pmat[dp, dp - 32] = 1.0
    consts = np.concatenate(
        [np.eye(128, dtype=np.float32), np.ones((128, 128), np.float32),
         pmat.T], axis=1).astype(NPBF16)

    # shared diagonal-block triangle mask, [k, q]: masked where q < k
    kk, qq = np.meshgrid(np.arange(128), np.arange(128), indexing="ij")
    tri = np.where(qq >= kk, np.float32(0.0), np.float32(NEG))
    tri = np.ascontiguousarray(tri.astype(np.float32))

    mask2d = np.ascontiguousarray(attention_mask[0, 0])   # [L(q), L(k)]
    if causal:
        mask_t_full = None
    else:
        mask_t_full = np.ascontiguousarray(mask2d.T.astype(np.float32))

    in_maps = []
    for c in range(NCORES):
        cols = []
        for j in range(HPC):
            h = c * HPC + j
            cols.append(w_qkv[:, h * HD:(h + 1) * HD])
        cols.append(w_qkv[:, q_pos + c * HD:q_pos + (c + 1) * HD])
        cols.append(w_qkv[:, kv_pos + c * HD:kv_pos + (c + 1) * HD])
        wqkv_c = np.ascontiguousarray(
            np.concatenate(cols, axis=1)).astype(NPBF16)             # [D, 768]
        wo_c = np.ascontiguousarray(
            w_o[c * HPC * HD:(c + 1) * HPC * HD, :]).astype(NPBF16)  # [512, D]
        m = {"xt": xt, "wqkv": wqkv_c, "wo": wo_c,
             "cos_e": cos_e, "sin_e": sin_e, "consts": consts, "tri_t": tri}
        if not causal:
            m["mask_t"] = mask_t_full
        in_maps.append(m)
    return in_maps


def _is_causal(mask2d):
    expected = np.where(
        np.tril(np.ones((L, L), dtype=bool)), np.float32(0.0), np.float32(NEG))
    return mask2d.shape == (L, L) and np.array_equal(mask2d, expected)


def kernel(x, attention_mask, cos, sin, w_qkv, w_o, _trace=False):
    from concourse.bass_utils import run_bass_kernel_spmd

    x = np.asarray(x, dtype=np.float32)
    attention_mask = np.asarray(attention_mask, dtype=np.float32)
    cos = np.asarray(cos, dtype=np.float32)
    sin = np.asarray(sin, dtype=np.float32)
    w_qkv = np.asarray(w_qkv, dtype=np.float32)
    w_o = np.asarray(w_o, dtype=np.float32)

    causal = _is_causal(attention_mask[0, 0])
    if causal not in _cache:
        _cache[causal] = _build(causal)
    nc = _cache[causal]

    in_maps = _host_inputs(x, attention_mask, cos, sin, w_qkv, w_o, causal)
    try:
        res = run_bass_kernel_spmd(nc, in_maps, list(range(NCORES)), trace=_trace)
    except Exception:
        # transient device errors (e.g. NRT_EXEC_UNIT_UNRECOVERABLE) usually
        # clear on retry
        res = run_bass_kernel_spmd(nc, in_maps, list(range(NCORES)), trace=_trace)
    out = np.zeros((L, D), np.float64)
    for c in range(NCORES):
        out += res.results[c]["out_p"].astype(np.float64)
    if _trace:
        kernel._last_exec_time_ns = res.exec_time_ns
    return out.astype(np.float32).reshape(B, L, D)


# revision 8
# speedup vs baseline: 1.1742x; 1.0638x over previous
"""Tensor-parallel attention kernel for Trainium2 (8 NeuronCores).

Problem: B=1, L=2048, D=4096, H=32 q-heads, KV=8 kv-heads, HD=128,
partial rotary ROT=64, causal additive mask, o-projection.

Sharding: TP-8 over heads. Core c owns q-heads 4c..4c+3 and kv-head c
(column shard of w_qkv), plus the matching row shard of w_o. Each core
computes a full [L, D] partial of the output; the host sums the 8
partials (the cross-core reduction of the row-sharded o-projection).

All on-chip data is bf16 (PSUM accumulation stays f32): halves HBM
traffic vs f32 and lets the PE use FWL weight loads. rel-err budget is
2e-2; bf16 end-to-end measures ~4e-3.

Everything runs in "transposed" orientation so every matmul contracts
over the partition dim with zero on-chip activation transposes:
  qkvT[col, L] = w_qkv.T @ x.T          (w stationary, xT streamed)
  rope:  qT' = qT * cosE + (P @ qT) * sinE   (P = rotate-half matrix on PE)
  ST[k, q]   = kT_tile.T @ qT            (one matmul per k-tile, K=HD=128)
  PT         = exp(ST + tri)             (exp batched over k-tile pairs)
  den[*, q]  = ones.T @ PT               (ones-matmul, accumulated over k)
  oT[d, q]   = V_tile.T @ PT             (V from a one-time PE transpose of vT)
  out[l, e]  = (oT/den).T @ w_o_shard    (partial; summed across cores on host)

Causality is exploited at 128-row k-strip granularity: strips fully
below the diagonal skip the mask entirely; diagonal strips compute only
the valid q-subrange and add a single shared [128,128] triangle mask.
"""

import sys

for _p in ("/opt/trn_rl_repo", "/root/.axon_site/_ro/trn_rl_repo"):
    if _p not in sys.path:
        sys.path.append(_p)

import numpy as np
import ml_dtypes

B, L, D = 1, 2048, 4096
H, KV, HD = 32, 8, 128
ROT = 64
SCALE = HD ** -0.5
NEG = -1e9
NCORES = 8
HPC = H // NCORES          # q-heads per core (4)
CPC = HPC * HD + 2 * HD    # w_qkv columns per core (768)
NDT = D // 128             # contraction tiles over D (32)
NKT = L // 128             # k tiles (16)
NJQ = L // 512             # 512-wide q blocks (4)
XBLK = 512                 # L-block width in the qkv phase
NLB = L // XBLK            # 4

NPBF16 = ml_dtypes.bfloat16

_cache = {}


def _build(causal: bool):
    import concourse.mybir as mybir
    import concourse.tile as tile
    from concourse import bacc

    F32 = mybir.dt.float32
    BF16 = mybir.dt.bfloat16
    EXP = mybir.ActivationFunctionType.Exp

    nc = bacc.Bacc("TRN2", target_bir_lowering=False, debug=False)

    # all big operands are pre-tiled on the host so every DMA line is a
    # long contiguous run (8-32KB): [partition, ...free] layouts
    xt_r = nc.dram_tensor("xt", [NLB, 128, NDT, XBLK], BF16,
                          kind="ExternalInput").ap()
    wqkv_r = nc.dram_tensor("wqkv", [128, NDT, CPC], BF16,
                            kind="ExternalInput").ap()
    wo_r = nc.dram_tensor("wo", [128, HPC, D], BF16, kind="ExternalInput").ap()
    tabs_t = nc.dram_tensor("tabs", [NLB, 128, 4, XBLK], F32,
                            kind="ExternalInput").ap()
    consts = nc.dram_tensor("consts", [128, 512], BF16,
                            kind="ExternalInput").ap()
    if not causal:
        mask_t = nc.dram_tensor("mask_t", [L, L], F32, kind="ExternalInput").ap()
    out_p = nc.dram_tensor("out_p", [L, D], BF16, kind="ExternalOutput").ap()

    with tile.TileContext(nc) as tc:
        with tc.tile_pool(name="persist", bufs=1) as persist:
            kt_sb = persist.tile([128, L], BF16, tag="kt")
            v_sb = persist.tile([128, NKT, 128], BF16, tag="v")
            qt_sb = persist.tile([128, HPC, L], BF16, tag="qt")
            wo_sb = persist.tile([128, HPC, D], BF16, tag="wo")
            cst = persist.tile([128, 512], BF16, tag="cst")
            nc.sync.dma_start(out=cst, in_=consts)
            ident = cst[:, 0:128]
            ones = cst[:, 128:256]
            pmat_t = cst[:, 256:384]
            tri01 = cst[:, 384:512]

            # ---------------- Phase 1: qkv projection + rope ----------------
            with tc.tile_pool(name="wq", bufs=1) as wqp, \
                 tc.tile_pool(name="xb", bufs=2) as xbp, \
                 tc.tile_pool(name="tabs", bufs=2) as tabs, \
                 tc.tile_pool(name="stage", bufs=3) as stage, \
                 tc.tile_pool(name="vtmp", bufs=2) as vtmp, \
                 tc.tile_pool(name="ps1", bufs=4, space="PSUM") as ps1, \
                 tc.tile_pool(name="psr", bufs=2, space="PSUM") as psr, \
                 tc.tile_pool(name="pst", bufs=2, space="PSUM") as pstp:
                wq_sb = wqp.tile([128, NDT, CPC], BF16)

                # weight DMA in dti-quarters on gpsimd; wo preload after
                for qtr in range(4):
                    qs_ = slice(8 * qtr, 8 * qtr + 8)
                    nc.gpsimd.dma_start(out=wq_sb[:, qs_, :], in_=wqkv_r[:, qs_, :])
                nc.gpsimd.dma_start(out=wo_sb, in_=wo_r)

                def rope_ct(lb, ct, acc, tb):
                    """Consume psum acc for column-group ct of L-block lb."""
                    ls = slice(lb * XBLK, (lb + 1) * XBLK)
                    if ct == 5:
                        # v: stage, then PE-transpose this block's k-tiles
                        vt_sb = vtmp.tile([128, XBLK], BF16, tag="vt")
                        nc.scalar.copy(out=vt_sb, in_=acc)
                        for kk in range(XBLK // 128):
                            i = lb * (XBLK // 128) + kk
                            tp = pstp.tile([128, 128], BF16, tag="vtp")
                            nc.tensor.transpose(
                                tp, vt_sb[:, kk * 128:(kk + 1) * 128], ident)
                            nc.vector.tensor_copy(v_sb[:, i, :], tp)
                        return
                    # rope for q (ct 0..3, scaled tables) and k (ct 4)
                    ti = 0 if ct < 4 else 1
                    s_sb = stage.tile([128, XBLK], BF16, tag="s_sb")
                    nc.scalar.copy(out=s_sb, in_=acc)
                    rot = psr.tile([128, XBLK], F32, tag="rot")
                    nc.tensor.matmul(out=rot, lhsT=pmat_t, rhs=s_sb,
                                     start=True, stop=True)
                    dst = qt_sb[:, ct, ls] if ct < 4 else kt_sb[:, ls]
                    m2 = stage.tile([128, XBLK], BF16, tag="m2")
                    # all-f32 inputs, bf16 outputs (cast on write)
                    nc.vector.tensor_mul(dst, acc, tb[:, ti, :])
                    nc.vector.tensor_mul(m2, rot, tb[:, 2 + ti, :])
                    nc.vector.tensor_add(dst, dst, m2)

                for lb in range(NLB):
                    xblk = xbp.tile([128, NDT, XBLK], BF16, tag="xblk")
                    if lb == 0:
                        for qtr in range(4):
                            qs_ = slice(8 * qtr, 8 * qtr + 8)
                            nc.sync.dma_start(out=xblk[:, qs_, :],
                                              in_=xt_r[lb][:, qs_, :])
                    else:
                        nc.sync.dma_start(out=xblk, in_=xt_r[lb])
                    tb = tabs.tile([128, 4, XBLK], F32, tag="tb")
                    nc.scalar.dma_start(out=tb, in_=tabs_t[lb])
                    if lb == 0:
                        # two passes of 3 column-groups, dti-quarter inner, so
                        # the first matmuls only wait on the first x/w quarter
                        accs = {}
                        for half in range(2):
                            cts = range(3 * half, 3 * half + 3)
                            for ct in cts:
                                accs[ct] = ps1.tile(
                                    [128, XBLK], F32, tag="acc",
                                    name=f"acc_l0_{ct}")
                            for qtr in range(4):
                                for ct in cts:
                                    for dti in range(8 * qtr, 8 * qtr + 8):
                                        nc.tensor.matmul(
                                            out=accs[ct],
                                            lhsT=wq_sb[:, dti,
                                                       ct * 128:(ct + 1) * 128],
                                            rhs=xblk[:, dti, :],
                                            start=(dti == 0),
                                            stop=(dti == NDT - 1))
                            for ct in cts:
                                rope_ct(lb, ct, accs[ct], tb)
                    else:
                        for ct in range(6):
                            acc = ps1.tile([128, XBLK], F32, tag="acc")
                            for dti in range(NDT):
                                nc.tensor.matmul(
                                    out=acc,
                                    lhsT=wq_sb[:, dti, ct * 128:(ct + 1) * 128],
                                    rhs=xblk[:, dti, :],
                                    start=(dti == 0), stop=(dti == NDT - 1))
                            rope_ct(lb, ct, acc, tb)

            # ---------------- Phases 2+3 ----------------
            late_cm = tc.tile_pool(name="late", bufs=1)
            late = late_cm.__enter__()
            otn_sb = late.tile([128, HPC, L], BF16, tag="otn")

            # ---------------- Phase 2: attention ----------------
            with tc.tile_pool(name="mb", bufs=2) as mbp, \
                 tc.tile_pool(name="pt", bufs=3) as ptp, \
                 tc.tile_pool(name="rdp", bufs=2) as rdp, \
                 tc.tile_pool(name="ps_st", bufs=2, space="PSUM") as ps_st, \
                 tc.tile_pool(name="ps_acc", bufs=2, space="PSUM") as ps_acc:
                for jq in range(NJQ):
                    qs = slice(jq * 512, (jq + 1) * 512)
                    if not causal:
                        mblk = mbp.tile([128, NKT, 512], F32, tag="mblk")
                        nc.scalar.dma_start(
                            out=mblk,
                            in_=mask_t[:, qs].rearrange("(kt p) q -> p kt q",
                                                        p=128))
                    for h in range(HPC):
                        den = ps_acc.tile([128, 512], F32, tag="den")
                        ot = ps_acc.tile([128, 512], F32, tag="ot")
                        # each pair entry: (pt2, i2 -> (ktile, dstart, width))
                        pend = None
                        first = [True]

                        def flush(pend):
                            if pend is None:
                                return
                            pt2, specs = pend
                            for i2, (i, ds, w) in enumerate(specs):
                                st_ = first[0] and i2 == 0
                                sp_ = (i == NKT - 1) if not causal else \
                                    (i == 4 * jq + 3)
                                nc.tensor.matmul(
                                    out=den[:, ds:ds + w], lhsT=ones[:, 0:128],
                                    rhs=pt2[:, i2, 0:w],
                                    start=st_, stop=sp_,
                                    skip_group_check=causal)
                                nc.tensor.matmul(
                                    out=ot[:, ds:ds + w], lhsT=v_sb[:, i, :],
                                    rhs=pt2[:, i2, 0:w],
                                    start=st_, stop=sp_,
                                    skip_group_check=causal)
                            first[0] = False

                        nfull = 4 * jq if causal else NKT
                        ndiag = 4 if causal else 0
                        npairs = (nfull + ndiag) // 2
                        for m in range(npairs):
                            st2 = ps_st.tile([128, 2, 512], F32, tag="st2")
                            pt2 = ptp.tile([128, 2, 512], BF16, tag="pt2")
                            specs = []
                            all_full = 2 * m + 1 < nfull
                            for i2 in range(2):
                                i = 2 * m + i2
                                if i < nfull:
                                    ds, w = 0, 512
                                    qsl = qs
                                else:
                                    d = i - nfull
                                    ds, w = 128 * d, 512 - 128 * d
                                    qsl = slice(jq * 512 + 128 * d,
                                                (jq + 1) * 512)
                                nc.tensor.matmul(
                                    out=st2[:, i2, 0:w],
                                    lhsT=kt_sb[:, i * 128:(i + 1) * 128],
                                    rhs=qt_sb[:, h, qsl],
                                    start=True, stop=True)
                                if not causal:
                                    nc.vector.tensor_add(
                                        st2[:, i2, :], st2[:, i2, :],
                                        mblk[:, i, :])
                                specs.append((i, ds, w))
                            if all_full or not causal:
                                nc.scalar.activation(pt2, st2, EXP)
                            else:
                                # diagonal strips: exp only the valid width,
                                # then zero the masked triangle in SBUF (keeps
                                # the Vector engine off the PSUM WAR chain)
                                for i2, (i, ds, w) in enumerate(specs):
                                    nc.scalar.activation(
                                        pt2[:, i2, 0:w], st2[:, i2, 0:w], EXP)
                                    nc.vector.tensor_mul(
                                        pt2[:, i2, 0:128], pt2[:, i2, 0:128],
                                        tri01)
                            flush(pend)
                            pend = (pt2, specs)
                        flush(pend)
                        rd = rdp.tile([128, 512], F32, tag="rd")
                        nc.vector.reciprocal_approx_fast(out=rd, in_=den)
                        nc.vector.tensor_mul(otn_sb[:, h, qs], ot, rd)

            # ---------------- Phase 3: o-projection ----------------
            with tc.tile_pool(name="ost", bufs=2) as ostp, \
                 tc.tile_pool(name="ps3", bufs=4, space="PSUM") as ps3:
                for lt in range(L // 128):
                    ost = ostp.tile([128, D], BF16, tag="ost")
                    for et in range(D // 512):
                        es = slice(et * 512, (et + 1) * 512)
                        acc = ps3.tile([128, 512], F32, tag="acc3")
                        for h in range(HPC):
                            nc.tensor.matmul(
                                out=acc,
                                lhsT=otn_sb[:, h, lt * 128:(lt + 1) * 128],
                                rhs=wo_sb[:, h, es],
                                start=(h == 0), stop=(h == HPC - 1))
                        if et % 2 == 0:
                            nc.vector.tensor_copy(ost[:, es], acc)
                        else:
                            nc.scalar.copy(out=ost[:, es], in_=acc)
                    nc.sync.dma_start(
                        out=out_p[lt * 128:(lt + 1) * 128, :], in_=ost)

            late_cm.__exit__(None, None, None)

    nc.compile()
    return nc


def _host_inputs(x, attention_mask, cos, sin, w_qkv, w_o, causal):
    """Build the 8 per-core input maps (bf16 data, host pre-tiled so every
    DMA line is a long contiguous run)."""
    q_pos = H * HD
    kv_pos = q_pos + KV * HD

    # x pre-tiled: [NLB, 128(p), NDT(dt), XBLK(j)] with
    # value = x[lb*XBLK + j, dt*128 + p]
    xt = np.ascontiguousarray(
        x[0].reshape(NLB, XBLK, NDT, 128).transpose(0, 3, 2, 1)
    ).astype(NPBF16)

    # rope tables pre-tiled: [NLB, 128, 4, XBLK], slots =
    # [cos_q(scale folded), cos_k, sin_q(scaled), sin_k]
    cos_t = cos.T.astype(np.float32)                      # [ROT, L]
    sin_t = sin.T.astype(np.float32)
    tab = np.zeros((4, 128, L), np.float32)
    tab[0, :ROT] = cos_t * SCALE
    tab[0, ROT:] = SCALE
    tab[1, :ROT] = cos_t
    tab[1, ROT:] = 1.0
    tab[2, :ROT] = sin_t * SCALE
    tab[3, :ROT] = sin_t
    tabs = np.ascontiguousarray(
        tab.reshape(4, 128, NLB, XBLK).transpose(2, 1, 0, 3))

    # consts [128, 512] = [identity | ones | pmat_t | tri01] (bf16)
    pmat = np.zeros((128, 128), np.float32)
    for dp in range(32):
        pmat[dp, dp + 32] = -1.0
    for dp in range(32, 64):
        pmat[dp, dp - 32] = 1.0
    # 0/1 triangle applied to exp(S) on diagonal blocks, [k, q]: keep q >= k
    kk, qq = np.meshgrid(np.arange(128), np.arange(128), indexing="ij")
    tri01 = (qq >= kk).astype(np.float32)
    consts = np.concatenate(
        [np.eye(128, dtype=np.float32), np.ones((128, 128), np.float32),
         pmat.T, tri01], axis=1).astype(NPBF16)

    mask2d = np.ascontiguousarray(attention_mask[0, 0])   # [L(q), L(k)]
    if causal:
        mask_t_full = None
    else:
        mask_t_full = np.ascontiguousarray(mask2d.T.astype(np.float32))

    in_maps = []
    for c in range(NCORES):
        cols = []
        for j in range(HPC):
            h = c * HPC + j
            cols.append(w_qkv[:, h * HD:(h + 1) * HD])
        cols.append(w_qkv[:, q_pos + c * HD:q_pos + (c + 1) * HD])
        cols.append(w_qkv[:, kv_pos + c * HD:kv_pos + (c + 1) * HD])
        wqkv_c = np.concatenate(cols, axis=1)                        # [D, 768]
        # pre-tile: [128(p), NDT(dt), CPC(c)]
        wqkv_c = np.ascontiguousarray(
            wqkv_c.reshape(NDT, 128, CPC).transpose(1, 0, 2)).astype(NPBF16)
        wo_c = w_o[c * HPC * HD:(c + 1) * HPC * HD, :]               # [512, D]
        # pre-tile: [128(p), HPC(h), D(e)]
        wo_c = np.ascontiguousarray(
            wo_c.reshape(HPC, 128, D).transpose(1, 0, 2)).astype(NPBF16)
        m = {"xt": xt, "wqkv": wqkv_c, "wo": wo_c,
             "tabs": tabs, "consts": consts}
        if not causal:
            m["mask_t"] = mask_t_full
        in_maps.append(m)
    return in_maps


def _is_causal(mask2d):
    expected = np.where(
        np.tril(np.ones((L, L), dtype=bool)), np.float32(0.0), np.float32(NEG))
    return mask2d.shape == (L, L) and np.array_equal(mask2d, expected)


def kernel(x, attention_mask, cos, sin, w_qkv, w_o, _trace=False):
    from concourse.bass_utils import run_bass_kernel_spmd

    x = np.asarray(x, dtype=np.float32)
    attention_mask = np.asarray(attention_mask, dtype=np.float32)
    cos = np.asarray(cos, dtype=np.float32)
    sin = np.asarray(sin, dtype=np.float32)
    w_qkv = np.asarray(w_qkv, dtype=np.float32)
    w_o = np.asarray(w_o, dtype=np.float32)

    causal = _is_causal(attention_mask[0, 0])
    if causal not in _cache:
        _cache[causal] = _build(causal)
    nc = _cache[causal]

    in_maps = _host_inputs(x, attention_mask, cos, sin, w_qkv, w_o, causal)
    try:
        res = run_bass_kernel_spmd(nc, in_maps, list(range(NCORES)), trace=_trace)
    except Exception:
        # transient device errors (e.g. NRT_EXEC_UNIT_UNRECOVERABLE) usually
        # clear on retry
        res = run_bass_kernel_spmd(nc, in_maps, list(range(NCORES)), trace=_trace)
    out = np.zeros((L, D), np.float64)
    for c in range(NCORES):
        out += res.results[c]["out_p"].astype(np.float64)
    if _trace:
        kernel._last_exec_time_ns = res.exec_time_ns
    return out.astype(np.float32).reshape(B, L, D)
